# revision 31
# baseline (speedup 1.0000x reference)
"""Trainium2 Bass kernel for nn_ETypeClusModel (ragged span max-pool + AE + cluster softmax).

Self-contained: hardcodes shapes. Shards the batch dim over 8 NeuronCores
(data parallel), replicates the AE weights + topic embedding, and gathers
p / per-row cosine distances back to the host (loss = mean of 128 values).

Layout strategy:
 - span max-pool: stream token rows [128, 1024], additive -1e30 mask via
   per-partition scalars, chunk-pair max, PE-transpose 128x128 blocks,
   free-dim reduce_max -> feat^T columns.
 - feat is re-assembled in natural [row, d] layout (8 transposes) where the
   zero-clamp, l2-norm, and cosine head are all per-partition ops.
 - MLP runs with activations as the stationary operand (lhsT [128, 16]) and
   weights as the moving operand (N up to 512), so each layer is a handful
   of wide matmuls instead of many N=16 ones. Bias+ReLU ride the
   inter-layer transposes on the scalar engine (per-partition bias).
"""

import os
import numpy as np

import concourse.bacc as bacc
import concourse.bass as bass
import concourse.mybir as mybir
import concourse.tile as tile
from concourse.bass_utils import run_bass_kernel_spmd

F32 = mybir.dt.float32
I32 = mybir.dt.int32
AF = mybir.ActivationFunctionType
OP = mybir.AluOpType
AX = mybir.AxisListType

B, L, D = 128, 256, 1024
NCORES = 8
BL = B // NCORES  # 16 rows per core
KTOP = 50
TEMP = 0.1
EPS = 1e-12
BIG = 1.0e30

# layer chain 1024->500->500->1000->100->1000->500->500->1024, padded to /128
CHAIN = [1024, 500, 500, 1000, 100, 1000, 500, 500, 1024]
PCH = [1024, 512, 512, 1024, 128, 1024, 512, 512, 1024]
RELU = [True, True, True, False, True, True, True, False]

LAST_EXEC_NS = None  # set after each kernel() call when tracing enabled
TRACE = bool(int(os.environ.get("BASS_KERNEL_TRACE", "0")))
FP32R = bool(int(os.environ.get("BASS_FP32R", "1")))


WDT = mybir.dt.float32r if FP32R else mybir.dt.float32


def _w(ap):
    """Bitcast a DRAM fp32 AP to the MLP operand dtype (float32r mode)."""
    return ap.bitcast(WDT) if FP32R else ap


def build_module():
    nc = bacc.Bacc("TRN2", target_bir_lowering=False)

    enc = nc.dram_tensor("enc", (BL, L, D), F32, kind="ExternalInput")
    vmask = nc.dram_tensor("vmask", (BL, L), I32, kind="ExternalInput")
    pspan = nc.dram_tensor("pspan", (BL, 2), I32, kind="ExternalInput")
    w_dram = [
        nc.dram_tensor(f"w{i}", (PCH[i], PCH[i + 1]), F32, kind="ExternalInput")
        for i in range(8)
    ]
    b_dram = [
        nc.dram_tensor(f"b{i}", (PCH[i + 1],), F32, kind="ExternalInput")
        for i in range(8)
    ]
    temb_d = nc.dram_tensor("temb", (KTOP, 100), F32, kind="ExternalInput")
    ident_d = nc.dram_tensor("ident", (128, 128), F32, kind="ExternalInput")

    p_out = nc.dram_tensor("p_out", (BL, KTOP), F32, kind="ExternalOutput")
    cos_out = nc.dram_tensor("cos_out", (BL, 1), F32, kind="ExternalOutput")

    with tile.TileContext(nc) as tc:
        with (
            tc.tile_pool(name="const", bufs=1) as cpool,
            tc.tile_pool(name="work", bufs=1) as wpool,
            tc.tile_pool(name="stream", bufs=3) as spool,
            tc.tile_pool(name="psum", bufs=1, space="PSUM") as ppool,
        ):
            # ---------------- constants needed early -----------------
            ident = cpool.tile([128, 128], F32, tag="ident")
            nc.sync.dma_start(ident, ident_d[:, :])
            ones1f = cpool.tile([1, 128], F32, tag="ones1f")
            nc.vector.memset(ones1f, 1.0)
            ones1 = cpool.tile([1, 128], WDT, tag="ones1")
            if FP32R:
                nc.vector.tensor_copy(ones1, ones1f)
            else:
                nc.vector.tensor_copy(ones1, ones1f)

            # ---------------- span / validity masks -----------------
            vm_i = wpool.tile([BL, L], I32, tag="vm_i")
            nc.sync.dma_start(vm_i, vmask[:, :])
            vm = wpool.tile([BL, L], F32, tag="vm")
            nc.vector.tensor_copy(vm, vm_i)
            zer = wpool.tile([BL, L], F32, tag="zer")
            nc.vector.memset(zer, 0.0)
            incl = wpool.tile([BL, L], F32, tag="incl")
            nc.vector.tensor_tensor_scan(incl, vm, zer, 0.0, OP.add, OP.add)
            rank = wpool.tile([BL, L], F32, tag="rank")
            nc.vector.tensor_sub(rank, incl, vm)

            ps_i = wpool.tile([BL, 2], I32, tag="ps_i")
            nc.sync.dma_start(ps_i, pspan[:, :])
            psf = wpool.tile([BL, 2], F32, tag="psf")
            nc.vector.tensor_copy(psf, ps_i)

            ge = wpool.tile([BL, L], F32, tag="ge")
            nc.vector.tensor_scalar(ge, rank, psf[:, 0:1], None, OP.is_ge)
            le = wpool.tile([BL, L], F32, tag="le")
            nc.vector.tensor_scalar(le, rank, psf[:, 1:2], None, OP.is_le)
            nc.vector.tensor_mul(ge, ge, le)
            nc.vector.tensor_mul(ge, ge, vm)
            am = wpool.tile([BL, L], F32, tag="am")
            nc.vector.tensor_scalar(am, ge, 1.0, BIG, OP.subtract, OP.mult)

            # zmask: 0 if span reaches past the valid count, else -BIG
            gz = wpool.tile([BL, 1], F32, tag="gz")
            nc.vector.tensor_tensor(gz, psf[:, 1:2], incl[:, L - 1 : L], OP.is_ge)
            zm = wpool.tile([BL, 1], F32, tag="zm")
            nc.vector.tensor_scalar(zm, gz, 1.0, BIG, OP.subtract, OP.mult)

            # PE warm-up: observe the ident DMA lane first.
            ptw = ppool.tile([2, 2], F32, tag="misc", bufs=2)
            nc.tensor.transpose(ptw, ident[:2, :2], ident[:2, :2])

            # transpose additive mask to [token, row] columns
            amT = []
            for c in range(2):
                pt = ppool.tile([128, BL], F32, tag="misc", bufs=2, name=f"ptam{c}")
                nc.tensor.transpose(pt, am[:, c * 128 : (c + 1) * 128], ident[:BL, :BL])
                s = cpool.tile([128, BL], F32, tag=f"amT{c}", name=f"amT{c}")
                nc.vector.tensor_copy(s, pt)
                amT.append(s)

            # ---------------- weights / topic constants (gpsimd ring, early) -------
            wt = []
            for i in range(8):
                nk = PCH[i] // 128
                no = PCH[i + 1]
                w = cpool.tile([128, nk * no], WDT, tag=f"w{i}", name=f"wt{i}")
                nc.gpsimd.dma_start(
                    w.rearrange("p (c n) -> p c n", n=no),
                    _w(w_dram[i].rearrange("(c p) n -> p c n", p=128)),
                )
                wt.append(w)
            bt = {}
            for i in (0, 1, 2, 4, 5, 6):
                nm = PCH[i + 1] // 128
                t = cpool.tile([128, nm], F32, tag=f"b{i}", name=f"bt{i}")
                nc.gpsimd.dma_start(t, b_dram[i].rearrange("(c p) -> p c", p=128))
                bt[i] = t
            brow = {}
            for i in (3, 7):
                no = PCH[i + 1]
                t = cpool.tile([1, no], WDT, tag=f"br{i}", name=f"brow{i}")
                nc.gpsimd.dma_start(t, _w(b_dram[i].rearrange("(a n) -> a n", a=1)))
                brow[i] = t
            temb = cpool.tile([KTOP, 100], F32, tag="temb")
            nc.gpsimd.dma_start(temb, temb_d[:, :])

            # ---------------- stream tokens, masked max-pool -----------------
            # Per row: mask+chunk-max in [token, d] layout, then fold the
            # 128 token-partitions into the free dim via a DVE 32x32 stream
            # transpose + 3D-AP reduce, one small PE transpose, a second
            # reduce, and a tiny flatten DMA into the natural feat row.
            fn = wpool.tile([BL, D], F32, tag="fn")
            for b in range(BL):
                e0 = spool.tile([128, D], F32, tag="e0", bufs=4)
                e1 = spool.tile([128, D], F32, tag="e1", bufs=4)
                nc.sync.dma_start(e0, enc[b, 0:128, :])
                nc.sync.dma_start(e1, enc[b, 128:256, :])
                # e1 += mask1 on ACT (engine balance), then fused (e0+mask0) max e1
                nc.scalar.activation(e1, e1, AF.Identity, bias=amT[1][:, b : b + 1])
                nc.vector.scalar_tensor_tensor(
                    e0, e0, amT[0][:, b : b + 1], e1, OP.add, OP.max
                )
                # fused 32x32 block transpose + max over jr: red[32jb+dr, db]
                red = spool.tile([128, 32], F32, tag="red", bufs=4, name="red")
                nc.vector.tensor_reduce(
                    red, e0.rearrange("p (db jr) -> p db jr", jr=32),
                    axis=AX.X, op=OP.max, apply_transpose=True,
                )
                # pred[db, 32jb+dr]
                pred = ppool.tile([32, 128], F32, tag="tr", bufs=4, name="pred")
                nc.tensor.transpose(pred, red, ident)
                # max over jb: mT[db, dr] = feat_b[32db+dr]
                mT = spool.tile([32, 32], F32, tag="mT", bufs=4, name="mT")
                nc.vector.tensor_reduce(
                    mT, pred.rearrange("p (jb dr) -> p dr jb", dr=32),
                    axis=AX.X, op=OP.max,
                )
                # flatten [32db, 32dr] -> natural feat row [1, 1024]
                nc.gpsimd.dma_start(
                    fn[b : b + 1, :].rearrange("a (db dr) -> a db dr", dr=32), mT
                )

            # ---------------- feat natural: clamp, l2norm ---------
            nc.vector.tensor_scalar(fn, fn, zm[:, 0:1], None, OP.max)
            fsq = wpool.tile([BL, D], F32, tag="fsq")
            n2 = wpool.tile([BL, 1], F32, tag="n2")
            nc.vector.tensor_mul(fsq, fn, fn)
            nc.vector.tensor_reduce(n2, fsq, axis=AX.X, op=OP.add)
            nr = wpool.tile([BL, 1], F32, tag="nr")
            nc.scalar.sqrt(nr, n2)
            nc.vector.tensor_scalar_max(nr, nr, EPS)
            inv = wpool.tile([BL, 1], F32, tag="inv")
            nc.vector.reciprocal(inv, nr)
            xnat = wpool.tile([BL, D], F32, tag="xnat")
            nc.vector.tensor_scalar_mul(xnat, fn, inv[:, 0:1])

            # x^T chunks for the MLP (stationary operand)
            actT = []
            for m in range(8):
                px = ppool.tile([128, BL], F32, tag="misc", bufs=2, name="pxT")
                nc.tensor.transpose(px, xnat[:, m * 128 : (m + 1) * 128], ident[:BL, :BL])
                s = spool.tile([128, BL], WDT, tag="actT_x", bufs=8, name=f"xTn{m}")
                nc.vector.tensor_copy(s, px)
                actT.append(s)

            # ---------------- autoencoder MLP (A-chain) -----------------
            znat = None
            hnat = None
            for i in range(8):
                nk = PCH[i] // 128
                no = PCH[i + 1]
                nat = wpool.tile([BL, no], F32, tag=f"nat{i}", name=f"nat{i}")
                for j in range((no + 511) // 512):
                    w_cols = min(512, no - j * 512)
                    ps = ppool.tile([BL, w_cols], F32, tag="mlp", bufs=2, name=f"psl{i}_{j}")
                    for c in range(nk):
                        o = c * no + j * 512
                        nc.tensor.matmul(
                            ps, actT[c], wt[i][:, o : o + w_cols],
                            start=(c == 0), stop=(c == nk - 1 and i not in (3, 7)),
                        )
                    if i in (3, 7):
                        # bias via K=1 ones-row matmul (natural-exit layers)
                        nc.tensor.matmul(
                            ps, ones1[:, :BL],
                            brow[i][:, j * 512 : j * 512 + w_cols],
                            start=False, stop=True,
                        )
                    nc.vector.tensor_copy(nat[:, j * 512 : j * 512 + w_cols], ps)
                if i == 3:
                    znat = nat
                if i == 7:
                    hnat = nat
                    break  # no transpose needed after the last layer
                nxt = []
                for c in range(no // 128):
                    pT = ppool.tile([128, BL], F32, tag="tr", bufs=4, name=f"pT{i}_{c}")
                    nc.tensor.transpose(
                        pT, nat[:, c * 128 : (c + 1) * 128], ident[:BL, :BL]
                    )
                    oa = spool.tile(
                        [128, BL], WDT, tag=f"actT{i}", bufs=no // 128, name=f"a{i}_{c}"
                    )
                    if i in (3, 7):
                        nc.vector.tensor_copy(oa, pT)
                    else:
                        nc.scalar.activation(oa, pT, AF.Relu, bias=bt[i][:, c : c + 1])
                    nxt.append(oa)
                actT = nxt

            # ---------------- cluster softmax head -----------------
            z2 = wpool.tile([BL, 1], F32, tag="z2")
            zsq = wpool.tile([BL, PCH[4]], F32, tag="zsq")
            nc.vector.tensor_mul(zsq, znat, znat)
            nc.vector.tensor_reduce(z2, zsq, axis=AX.X, op=OP.add)
            znr = wpool.tile([BL, 1], F32, tag="znr")
            nc.scalar.sqrt(znr, z2)
            nc.vector.tensor_scalar_max(znr, znr, EPS)
            zinv = wpool.tile([BL, 1], F32, tag="zinv")
            nc.vector.reciprocal(zinv, znr)
            zn = wpool.tile([BL, PCH[4]], F32, tag="zn")
            nc.vector.tensor_scalar_mul(zn, znat, zinv[:, 0:1])
            pzT = ppool.tile([PCH[4], BL], F32, tag="misc", bufs=2)
            nc.tensor.transpose(pzT, zn, ident[:BL, :BL])
            znT = wpool.tile([PCH[4], BL], F32, tag="znT")
            nc.vector.tensor_copy(znT, pzT)

            tsq = wpool.tile([KTOP, 100], F32, tag="tsq")
            nc.vector.tensor_mul(tsq, temb, temb)
            tn2 = wpool.tile([KTOP, 1], F32, tag="tn2")
            nc.vector.tensor_reduce(tn2, tsq, axis=AX.X, op=OP.add)
            tnr = wpool.tile([KTOP, 1], F32, tag="tnr")
            nc.scalar.sqrt(tnr, tn2)
            nc.vector.tensor_scalar_max(tnr, tnr, EPS)
            tiv = wpool.tile([KTOP, 1], F32, tag="tiv")
            nc.vector.reciprocal(tiv, tnr)
            tn = wpool.tile([KTOP, 100], F32, tag="tn")
            nc.vector.tensor_scalar_mul(tn, temb, tiv)
            ptt = ppool.tile([100, KTOP], F32, tag="misc", bufs=2)
            nc.tensor.transpose(ptt, tn, ident[:KTOP, :KTOP])
            tnT = wpool.tile([100, KTOP], F32, tag="tnT")
            nc.vector.tensor_copy(tnT, ptt)

            psl = ppool.tile([BL, KTOP], F32, tag="misc", bufs=2)
            nc.tensor.matmul(psl, znT[0:100, :], tnT, start=True, stop=True)
            mx = wpool.tile([BL, 1], F32, tag="mx")
            nc.vector.tensor_reduce(mx, psl, axis=AX.X, op=OP.max)
            mxs = wpool.tile([BL, 1], F32, tag="mxs")
            nc.vector.tensor_scalar_mul(mxs, mx, -1.0 / TEMP)
            ex = wpool.tile([BL, KTOP], F32, tag="ex")
            nc.scalar.activation(ex, psl, AF.Exp, bias=mxs, scale=1.0 / TEMP)
            sm = wpool.tile([BL, 1], F32, tag="sm")
            nc.vector.tensor_reduce(sm, ex, axis=AX.X, op=OP.add)
            rs = wpool.tile([BL, 1], F32, tag="rs")
            nc.vector.reciprocal(rs, sm)
            pp = wpool.tile([BL, KTOP], F32, tag="pp")
            nc.vector.tensor_scalar_mul(pp, ex, rs)
            nc.sync.dma_start(p_out[:, :], pp)

            # ---------------- cosine pretrain loss -----------------
            hx = wpool.tile([BL, D], F32, tag="hx")
            s1 = wpool.tile([BL, 1], F32, tag="s1")
            nc.vector.tensor_mul(hx, hnat, xnat)
            nc.vector.tensor_reduce(s1, hx, axis=AX.X, op=OP.add)
            hh = wpool.tile([BL, D], F32, tag="hh")
            n2h = wpool.tile([BL, 1], F32, tag="n2h")
            nc.vector.tensor_mul(hh, hnat, hnat)
            nc.vector.tensor_reduce(n2h, hh, axis=AX.X, op=OP.add)
            hnr = wpool.tile([BL, 1], F32, tag="hnr")
            nc.scalar.sqrt(hnr, n2h)
            nc.vector.tensor_scalar_max(hnr, hnr, EPS)
            hinv = wpool.tile([BL, 1], F32, tag="hinv")
            nc.vector.reciprocal(hinv, hnr)
            ct = wpool.tile([BL, 1], F32, tag="ct")
            nc.vector.tensor_mul(ct, s1, hinv)
            cosd = wpool.tile([BL, 1], F32, tag="cosd")
            nc.vector.tensor_scalar(cosd, ct, -1.0, 1.0, OP.mult, OP.add)
            nc.sync.dma_start(cos_out[:, :], cosd)

    nc.compile()
    return nc


def _pad_weights(enc_W, enc_b, dec_W, dec_b):
    Ws = [np.asarray(w, np.float32) for w in list(enc_W) + list(dec_W)]
    bs = [np.asarray(b, np.float32) for b in list(enc_b) + list(dec_b)]
    pw, pb = [], []
    for i in range(8):
        w = np.zeros((PCH[i], PCH[i + 1]), np.float32)
        w[: Ws[i].shape[0], : Ws[i].shape[1]] = Ws[i]
        b = np.zeros((PCH[i + 1],), np.float32)
        b[: bs[i].shape[0]] = bs[i]
        pw.append(w)
        pb.append(b)
    return pw, pb


def kernel(encoder_layers, valid_mask, pos_span, mask_span,
           enc_W, enc_b, dec_W, dec_b, topic_emb):
    global LAST_EXEC_NS
    enc = np.ascontiguousarray(np.asarray(encoder_layers, np.float32))
    vm = np.ascontiguousarray(np.asarray(valid_mask, np.int32))
    ps = np.ascontiguousarray(np.asarray(pos_span, np.int32))
    te = np.ascontiguousarray(np.asarray(topic_emb, np.float32))
    pw, pb = _pad_weights(enc_W, enc_b, dec_W, dec_b)
    ident = np.eye(128, dtype=np.float32)

    nc = build_module()

    in_maps = []
    for c in range(NCORES):
        sl = slice(c * BL, (c + 1) * BL)
        m = {"enc": enc[sl], "vmask": vm[sl], "pspan": ps[sl],
             "temb": te, "ident": ident}
        for i in range(8):
            m[f"w{i}"] = pw[i]
            m[f"b{i}"] = pb[i]
        in_maps.append(m)

    res = run_bass_kernel_spmd(
        nc, in_maps, core_ids=list(range(NCORES)), trace=TRACE,
    )
    LAST_EXEC_NS = res.exec_time_ns

    p = np.concatenate([r["p_out"] for r in res.results], axis=0)
    cos = np.concatenate([r["cos_out"] for r in res.results], axis=0).reshape(-1)
    loss = np.float32(cos.mean())
    return loss, p.astype(np.float32)


# revision 32
# speedup vs baseline: 1.1956x; 1.1956x over previous
"""Trainium2 Bass kernel for nn_ETypeClusModel (ragged span max-pool + AE + cluster softmax).

Self-contained: hardcodes shapes. Shards the batch dim over 8 NeuronCores
(data parallel), replicates the AE weights + topic embedding, and gathers
p / per-row cosine distances back to the host (loss = mean of 128 values).

Layout strategy:
 - span max-pool: stream token rows [128, 1024], additive -1e30 mask via
   per-partition scalars, chunk-pair max, PE-transpose 128x128 blocks,
   free-dim reduce_max -> feat^T columns.
 - feat is re-assembled in natural [row, d] layout (8 transposes) where the
   zero-clamp, l2-norm, and cosine head are all per-partition ops.
 - MLP runs with activations as the stationary operand (lhsT [128, 16]) and
   weights as the moving operand (N up to 512), so each layer is a handful
   of wide matmuls instead of many N=16 ones. Bias+ReLU ride the
   inter-layer transposes on the scalar engine (per-partition bias).
"""

import os
import numpy as np

import concourse.bacc as bacc
import concourse.bass as bass
import concourse.mybir as mybir
import concourse.tile as tile
from concourse.bass_utils import run_bass_kernel_spmd

F32 = mybir.dt.float32
I32 = mybir.dt.int32
AF = mybir.ActivationFunctionType
OP = mybir.AluOpType
AX = mybir.AxisListType

B, L, D = 128, 256, 1024
NCORES = 8
BL = B // NCORES  # 16 rows per core
KTOP = 50
TEMP = 0.1
EPS = 1e-12
BIG = 1.0e30

# layer chain 1024->500->500->1000->100->1000->500->500->1024, padded to /128
CHAIN = [1024, 500, 500, 1000, 100, 1000, 500, 500, 1024]
PCH = [1024, 512, 512, 1024, 128, 1024, 512, 512, 1024]
RELU = [True, True, True, False, True, True, True, False]

LAST_EXEC_NS = None  # set after each kernel() call when tracing enabled
TRACE = bool(int(os.environ.get("BASS_KERNEL_TRACE", "0")))
FP32R = bool(int(os.environ.get("BASS_FP32R", "1")))


WDT = mybir.dt.float32r if FP32R else mybir.dt.float32


def _w(ap):
    """Bitcast a DRAM fp32 AP to the MLP operand dtype (float32r mode)."""
    return ap.bitcast(WDT) if FP32R else ap


def build_module():
    nc = bacc.Bacc("TRN2", target_bir_lowering=False)

    enc = nc.dram_tensor("enc", (BL, L, D), F32, kind="ExternalInput")
    vmask = nc.dram_tensor("vmask", (BL, L), I32, kind="ExternalInput")
    pspan = nc.dram_tensor("pspan", (BL, 2), I32, kind="ExternalInput")
    w_dram = [
        nc.dram_tensor(f"w{i}", (PCH[i], PCH[i + 1]), F32, kind="ExternalInput")
        for i in range(8)
    ]
    b_dram = [
        nc.dram_tensor(f"b{i}", (PCH[i + 1],), F32, kind="ExternalInput")
        for i in range(8)
    ]
    temb_d = nc.dram_tensor("temb", (KTOP, 100), F32, kind="ExternalInput")
    ident_d = nc.dram_tensor("ident", (128, 128), F32, kind="ExternalInput")

    p_out = nc.dram_tensor("p_out", (BL, KTOP), F32, kind="ExternalOutput")
    cos_out = nc.dram_tensor("cos_out", (BL, 1), F32, kind="ExternalOutput")

    with tile.TileContext(nc) as tc:
        with (
            tc.tile_pool(name="const", bufs=1) as cpool,
            tc.tile_pool(name="work", bufs=1) as wpool,
            tc.tile_pool(name="stream", bufs=3) as spool,
            tc.tile_pool(name="psum", bufs=1, space="PSUM") as ppool,
        ):
            # ---------------- constants needed early -----------------
            ident = cpool.tile([128, 128], F32, tag="ident")
            nc.sync.dma_start(ident, ident_d[:, :])
            ones1f = cpool.tile([1, 128], F32, tag="ones1f")
            nc.vector.memset(ones1f, 1.0)
            ones1 = cpool.tile([1, 128], WDT, tag="ones1")
            if FP32R:
                nc.vector.tensor_copy(ones1, ones1f)
            else:
                nc.vector.tensor_copy(ones1, ones1f)

            # ---------------- span / validity masks -----------------
            vm_i = wpool.tile([BL, L], I32, tag="vm_i")
            nc.sync.dma_start(vm_i, vmask[:, :])
            vm = wpool.tile([BL, L], F32, tag="vm")
            nc.vector.tensor_copy(vm, vm_i)
            zer = wpool.tile([BL, L], F32, tag="zer")
            nc.vector.memset(zer, 0.0)
            incl = wpool.tile([BL, L], F32, tag="incl")
            nc.vector.tensor_tensor_scan(incl, vm, zer, 0.0, OP.add, OP.add)
            rank = wpool.tile([BL, L], F32, tag="rank")
            nc.vector.tensor_sub(rank, incl, vm)

            ps_i = wpool.tile([BL, 2], I32, tag="ps_i")
            nc.sync.dma_start(ps_i, pspan[:, :])
            psf = wpool.tile([BL, 2], F32, tag="psf")
            nc.vector.tensor_copy(psf, ps_i)

            ge = wpool.tile([BL, L], F32, tag="ge")
            nc.vector.tensor_scalar(ge, rank, psf[:, 0:1], None, OP.is_ge)
            le = wpool.tile([BL, L], F32, tag="le")
            nc.vector.tensor_scalar(le, rank, psf[:, 1:2], None, OP.is_le)
            nc.vector.tensor_mul(ge, ge, le)
            nc.vector.tensor_mul(ge, ge, vm)
            am = wpool.tile([BL, L], F32, tag="am")
            nc.vector.tensor_scalar(am, ge, 1.0, BIG, OP.subtract, OP.mult)

            # zmask: 0 if span reaches past the valid count, else -BIG
            gz = wpool.tile([BL, 1], F32, tag="gz")
            nc.vector.tensor_tensor(gz, psf[:, 1:2], incl[:, L - 1 : L], OP.is_ge)
            zm = wpool.tile([BL, 1], F32, tag="zm")
            nc.vector.tensor_scalar(zm, gz, 1.0, BIG, OP.subtract, OP.mult)

            # PE warm-up: observe the ident DMA lane first.
            ptw = ppool.tile([2, 2], F32, tag="misc", bufs=2)
            nc.tensor.transpose(ptw, ident[:2, :2], ident[:2, :2])

            # transpose additive mask to [token, row] columns
            amT = []
            for c in range(2):
                pt = ppool.tile([128, BL], F32, tag="misc", bufs=2, name=f"ptam{c}")
                nc.tensor.transpose(pt, am[:, c * 128 : (c + 1) * 128], ident[:BL, :BL])
                s = cpool.tile([128, BL], F32, tag=f"amT{c}", name=f"amT{c}")
                nc.vector.tensor_copy(s, pt)
                amT.append(s)

            # ---------------- stream tokens, masked max-pool -----------------
            # Per row: mask+chunk-max in [token, d] layout, then fold the
            # 128 token-partitions into the free dim via a DVE 32x32 stream
            # transpose + 3D-AP reduce, one small PE transpose, a second
            # reduce, and a tiny flatten DMA into the natural feat row.
            fn = wpool.tile([BL, D], F32, tag="fn")
            for b in range(BL):
                ee = spool.tile([128, 2 * D], F32, tag="ee", bufs=4)
                nc.sync.dma_start(
                    ee.rearrange("p (c m) -> p c m", m=D),
                    enc[b].rearrange("(c p) m -> p c m", p=128),
                )
                e0 = ee[:, 0:D]
                e1 = ee[:, D : 2 * D]
                # e1 += mask1 on ACT (engine balance), then fused (e0+mask0) max e1
                nc.scalar.activation(e1, e1, AF.Identity, bias=amT[1][:, b : b + 1])
                nc.vector.scalar_tensor_tensor(
                    e0, e0, amT[0][:, b : b + 1], e1, OP.add, OP.max
                )
                # fused 32x32 block transpose + max over jr: red[32jb+dr, db]
                red = spool.tile([128, 32], F32, tag="red", bufs=4, name="red")
                nc.vector.tensor_reduce(
                    red, e0.rearrange("p (db jr) -> p db jr", jr=32),
                    axis=AX.X, op=OP.max, apply_transpose=True,
                )
                # pred[db, 32jb+dr]
                pred = ppool.tile([32, 128], F32, tag="tr", bufs=4, name="pred")
                nc.tensor.transpose(pred, red, ident)
                # max over jb: mT[db, dr] = feat_b[32db+dr]
                mT = spool.tile([32, 32], F32, tag="mT", bufs=4, name="mT")
                nc.vector.tensor_reduce(
                    mT, pred.rearrange("p (jb dr) -> p dr jb", dr=32),
                    axis=AX.X, op=OP.max,
                )
                # flatten [32db, 32dr] -> natural feat row [1, 1024]
                nc.gpsimd.dma_start(
                    fn[b : b + 1, :].rearrange("a (db dr) -> a db dr", dr=32), mT
                )

            # ---------------- weights / topic constants (after stream) -------
            wt = []
            for i in range(8):
                nk = PCH[i] // 128
                no = PCH[i + 1]
                w = cpool.tile([128, nk * no], WDT, tag=f"w{i}", name=f"wt{i}")
                nc.sync.dma_start(
                    w.rearrange("p (c n) -> p c n", n=no),
                    _w(w_dram[i].rearrange("(c p) n -> p c n", p=128)),
                )
                wt.append(w)
            bt = {}
            for i in (0, 1, 2, 4, 5, 6):
                nm = PCH[i + 1] // 128
                t = cpool.tile([128, nm], F32, tag=f"b{i}", name=f"bt{i}")
                nc.sync.dma_start(t, b_dram[i].rearrange("(c p) -> p c", p=128))
                bt[i] = t
            brow = {}
            for i in (3, 7):
                no = PCH[i + 1]
                t = cpool.tile([1, no], WDT, tag=f"br{i}", name=f"brow{i}")
                nc.sync.dma_start(t, _w(b_dram[i].rearrange("(a n) -> a n", a=1)))
                brow[i] = t
            temb = cpool.tile([KTOP, 100], F32, tag="temb")
            nc.sync.dma_start(temb, temb_d[:, :])

            # ---------------- feat natural: clamp, l2norm ---------
            nc.vector.tensor_scalar(fn, fn, zm[:, 0:1], None, OP.max)
            fsq = wpool.tile([BL, D], F32, tag="fsq")
            n2 = wpool.tile([BL, 1], F32, tag="n2")
            nc.vector.tensor_mul(fsq, fn, fn)
            nc.vector.tensor_reduce(n2, fsq, axis=AX.X, op=OP.add)
            nr = wpool.tile([BL, 1], F32, tag="nr")
            nc.scalar.sqrt(nr, n2)
            nc.vector.tensor_scalar_max(nr, nr, EPS)
            inv = wpool.tile([BL, 1], F32, tag="inv")
            nc.vector.reciprocal(inv, nr)
            xnat = wpool.tile([BL, D], F32, tag="xnat")
            nc.vector.tensor_scalar_mul(xnat, fn, inv[:, 0:1])

            # x^T chunks for the MLP (stationary operand)
            actT = []
            for m in range(8):
                px = ppool.tile([128, BL], F32, tag="misc", bufs=2, name="pxT")
                nc.tensor.transpose(px, xnat[:, m * 128 : (m + 1) * 128], ident[:BL, :BL])
                s = spool.tile([128, BL], WDT, tag="actT_x", bufs=8, name=f"xTn{m}")
                nc.vector.tensor_copy(s, px)
                actT.append(s)

            # ---------------- autoencoder MLP (A-chain) -----------------
            znat = None
            hnat = None
            for i in range(8):
                nk = PCH[i] // 128
                no = PCH[i + 1]
                nat = wpool.tile([BL, no], F32, tag=f"nat{i}", name=f"nat{i}")
                for j in range((no + 511) // 512):
                    w_cols = min(512, no - j * 512)
                    ps = ppool.tile([BL, w_cols], F32, tag="mlp", bufs=2, name=f"psl{i}_{j}")
                    for c in range(nk):
                        o = c * no + j * 512
                        nc.tensor.matmul(
                            ps, actT[c], wt[i][:, o : o + w_cols],
                            start=(c == 0), stop=(c == nk - 1 and i not in (3, 7)),
                        )
                    if i in (3, 7):
                        # bias via K=1 ones-row matmul (natural-exit layers)
                        nc.tensor.matmul(
                            ps, ones1[:, :BL],
                            brow[i][:, j * 512 : j * 512 + w_cols],
                            start=False, stop=True,
                        )
                    nc.vector.tensor_copy(nat[:, j * 512 : j * 512 + w_cols], ps)
                if i == 3:
                    znat = nat
                if i == 7:
                    hnat = nat
                    break  # no transpose needed after the last layer
                nxt = []
                for c in range(no // 128):
                    pT = ppool.tile([128, BL], F32, tag="tr", bufs=4, name=f"pT{i}_{c}")
                    nc.tensor.transpose(
                        pT, nat[:, c * 128 : (c + 1) * 128], ident[:BL, :BL]
                    )
                    oa = spool.tile(
                        [128, BL], WDT, tag=f"actT{i}", bufs=no // 128, name=f"a{i}_{c}"
                    )
                    if i in (3, 7):
                        nc.vector.tensor_copy(oa, pT)
                    else:
                        nc.scalar.activation(oa, pT, AF.Relu, bias=bt[i][:, c : c + 1])
                    nxt.append(oa)
                actT = nxt

            # ---------------- cluster softmax head -----------------
            z2 = wpool.tile([BL, 1], F32, tag="z2")
            zsq = wpool.tile([BL, PCH[4]], F32, tag="zsq")
            nc.vector.tensor_mul(zsq, znat, znat)
            nc.vector.tensor_reduce(z2, zsq, axis=AX.X, op=OP.add)
            znr = wpool.tile([BL, 1], F32, tag="znr")
            nc.scalar.sqrt(znr, z2)
            nc.vector.tensor_scalar_max(znr, znr, EPS)
            zinv = wpool.tile([BL, 1], F32, tag="zinv")
            nc.vector.reciprocal(zinv, znr)
            zn = wpool.tile([BL, PCH[4]], F32, tag="zn")
            nc.vector.tensor_scalar_mul(zn, znat, zinv[:, 0:1])
            pzT = ppool.tile([PCH[4], BL], F32, tag="misc", bufs=2)
            nc.tensor.transpose(pzT, zn, ident[:BL, :BL])
            znT = wpool.tile([PCH[4], BL], F32, tag="znT")
            nc.vector.tensor_copy(znT, pzT)

            tsq = wpool.tile([KTOP, 100], F32, tag="tsq")
            nc.vector.tensor_mul(tsq, temb, temb)
            tn2 = wpool.tile([KTOP, 1], F32, tag="tn2")
            nc.vector.tensor_reduce(tn2, tsq, axis=AX.X, op=OP.add)
            tnr = wpool.tile([KTOP, 1], F32, tag="tnr")
            nc.scalar.sqrt(tnr, tn2)
            nc.vector.tensor_scalar_max(tnr, tnr, EPS)
            tiv = wpool.tile([KTOP, 1], F32, tag="tiv")
            nc.vector.reciprocal(tiv, tnr)
            tn = wpool.tile([KTOP, 100], F32, tag="tn")
            nc.vector.tensor_scalar_mul(tn, temb, tiv)
            ptt = ppool.tile([100, KTOP], F32, tag="misc", bufs=2)
            nc.tensor.transpose(ptt, tn, ident[:KTOP, :KTOP])
            tnT = wpool.tile([100, KTOP], F32, tag="tnT")
            nc.vector.tensor_copy(tnT, ptt)

            psl = ppool.tile([BL, KTOP], F32, tag="misc", bufs=2)
            nc.tensor.matmul(psl, znT[0:100, :], tnT, start=True, stop=True)
            mx = wpool.tile([BL, 1], F32, tag="mx")
            nc.vector.tensor_reduce(mx, psl, axis=AX.X, op=OP.max)
            mxs = wpool.tile([BL, 1], F32, tag="mxs")
            nc.vector.tensor_scalar_mul(mxs, mx, -1.0 / TEMP)
            ex = wpool.tile([BL, KTOP], F32, tag="ex")
            nc.scalar.activation(ex, psl, AF.Exp, bias=mxs, scale=1.0 / TEMP)
            sm = wpool.tile([BL, 1], F32, tag="sm")
            nc.vector.tensor_reduce(sm, ex, axis=AX.X, op=OP.add)
            rs = wpool.tile([BL, 1], F32, tag="rs")
            nc.vector.reciprocal(rs, sm)
            pp = wpool.tile([BL, KTOP], F32, tag="pp")
            nc.vector.tensor_scalar_mul(pp, ex, rs)
            nc.sync.dma_start(p_out[:, :], pp)

            # ---------------- cosine pretrain loss -----------------
            hx = wpool.tile([BL, D], F32, tag="hx")
            s1 = wpool.tile([BL, 1], F32, tag="s1")
            nc.vector.tensor_mul(hx, hnat, xnat)
            nc.vector.tensor_reduce(s1, hx, axis=AX.X, op=OP.add)
            hh = wpool.tile([BL, D], F32, tag="hh")
            n2h = wpool.tile([BL, 1], F32, tag="n2h")
            nc.vector.tensor_mul(hh, hnat, hnat)
            nc.vector.tensor_reduce(n2h, hh, axis=AX.X, op=OP.add)
            hnr = wpool.tile([BL, 1], F32, tag="hnr")
            nc.scalar.sqrt(hnr, n2h)
            nc.vector.tensor_scalar_max(hnr, hnr, EPS)
            hinv = wpool.tile([BL, 1], F32, tag="hinv")
            nc.vector.reciprocal(hinv, hnr)
            ct = wpool.tile([BL, 1], F32, tag="ct")
            nc.vector.tensor_mul(ct, s1, hinv)
            cosd = wpool.tile([BL, 1], F32, tag="cosd")
            nc.vector.tensor_scalar(cosd, ct, -1.0, 1.0, OP.mult, OP.add)
            nc.sync.dma_start(cos_out[:, :], cosd)

    nc.compile()
    return nc


def _pad_weights(enc_W, enc_b, dec_W, dec_b):
    Ws = [np.asarray(w, np.float32) for w in list(enc_W) + list(dec_W)]
    bs = [np.asarray(b, np.float32) for b in list(enc_b) + list(dec_b)]
    pw, pb = [], []
    for i in range(8):
        w = np.zeros((PCH[i], PCH[i + 1]), np.float32)
        w[: Ws[i].shape[0], : Ws[i].shape[1]] = Ws[i]
        b = np.zeros((PCH[i + 1],), np.float32)
        b[: bs[i].shape[0]] = bs[i]
        pw.append(w)
        pb.append(b)
    return pw, pb


def kernel(encoder_layers, valid_mask, pos_span, mask_span,
           enc_W, enc_b, dec_W, dec_b, topic_emb):
    global LAST_EXEC_NS
    enc = np.ascontiguousarray(np.asarray(encoder_layers, np.float32))
    vm = np.ascontiguousarray(np.asarray(valid_mask, np.int32))
    ps = np.ascontiguousarray(np.asarray(pos_span, np.int32))
    te = np.ascontiguousarray(np.asarray(topic_emb, np.float32))
    pw, pb = _pad_weights(enc_W, enc_b, dec_W, dec_b)
    ident = np.eye(128, dtype=np.float32)

    nc = build_module()

    in_maps = []
    for c in range(NCORES):
        sl = slice(c * BL, (c + 1) * BL)
        m = {"enc": enc[sl], "vmask": vm[sl], "pspan": ps[sl],
             "temb": te, "ident": ident}
        for i in range(8):
            m[f"w{i}"] = pw[i]
            m[f"b{i}"] = pb[i]
        in_maps.append(m)

    res = run_bass_kernel_spmd(
        nc, in_maps, core_ids=list(range(NCORES)), trace=TRACE,
    )
    LAST_EXEC_NS = res.exec_time_ns

    p = np.concatenate([r["p_out"] for r in res.results], axis=0)
    cos = np.concatenate([r["cos_out"] for r in res.results], axis=0).reshape(-1)
    loss = np.float32(cos.mean())
    return loss, p.astype(np.float32)


# revision 33
# speedup vs baseline: 1.2353x; 1.0332x over previous
"""Trainium2 Bass kernel for nn_ETypeClusModel (ragged span max-pool + AE + cluster softmax).

Self-contained: hardcodes shapes. Shards the batch dim over 8 NeuronCores
(data parallel), replicates the AE weights + topic embedding, and gathers
p / per-row cosine distances back to the host (loss = mean of 128 values).

Layout strategy:
 - span max-pool: stream token rows [128, 1024], additive -1e30 mask via
   per-partition scalars, chunk-pair max, PE-transpose 128x128 blocks,
   free-dim reduce_max -> feat^T columns.
 - feat is re-assembled in natural [row, d] layout (8 transposes) where the
   zero-clamp, l2-norm, and cosine head are all per-partition ops.
 - MLP runs with activations as the stationary operand (lhsT [128, 16]) and
   weights as the moving operand (N up to 512), so each layer is a handful
   of wide matmuls instead of many N=16 ones. Bias+ReLU ride the
   inter-layer transposes on the scalar engine (per-partition bias).
"""

import os
import numpy as np

import concourse.bacc as bacc
import concourse.bass as bass
import concourse.mybir as mybir
import concourse.tile as tile
from concourse.bass_utils import run_bass_kernel_spmd

F32 = mybir.dt.float32
I32 = mybir.dt.int32
AF = mybir.ActivationFunctionType
OP = mybir.AluOpType
AX = mybir.AxisListType

B, L, D = 128, 256, 1024
NCORES = 8
BL = B // NCORES  # 16 rows per core
KTOP = 50
TEMP = 0.1
EPS = 1e-12
BIG = 1.0e30

# layer chain 1024->500->500->1000->100->1000->500->500->1024, padded to /128
CHAIN = [1024, 500, 500, 1000, 100, 1000, 500, 500, 1024]
PCH = [1024, 512, 512, 1024, 128, 1024, 512, 512, 1024]
RELU = [True, True, True, False, True, True, True, False]

LAST_EXEC_NS = None  # set after each kernel() call when tracing enabled
TRACE = bool(int(os.environ.get("BASS_KERNEL_TRACE", "0")))
FP32R = bool(int(os.environ.get("BASS_FP32R", "1")))


WDT = mybir.dt.float32r if FP32R else mybir.dt.float32


def _w(ap):
    """Bitcast a DRAM fp32 AP to the MLP operand dtype (float32r mode)."""
    return ap.bitcast(WDT) if FP32R else ap


def build_module():
    nc = bacc.Bacc("TRN2", target_bir_lowering=False)

    enc = nc.dram_tensor("enc", (BL, L, D), F32, kind="ExternalInput")
    vmask = nc.dram_tensor("vmask", (BL, L), I32, kind="ExternalInput")
    pspan = nc.dram_tensor("pspan", (BL, 2), I32, kind="ExternalInput")
    w_dram = [
        nc.dram_tensor(f"w{i}", (PCH[i], PCH[i + 1]), F32, kind="ExternalInput")
        for i in range(8)
    ]
    b_dram = [
        nc.dram_tensor(f"b{i}", (PCH[i + 1],), F32, kind="ExternalInput")
        for i in range(8)
    ]
    temb_d = nc.dram_tensor("temb", (KTOP, 100), F32, kind="ExternalInput")
    ident_d = nc.dram_tensor("ident", (128, 128), F32, kind="ExternalInput")

    p_out = nc.dram_tensor("p_out", (BL, KTOP), F32, kind="ExternalOutput")
    cos_out = nc.dram_tensor("cos_out", (BL, 1), F32, kind="ExternalOutput")

    with tile.TileContext(nc) as tc:
        with (
            tc.tile_pool(name="const", bufs=1) as cpool,
            tc.tile_pool(name="work", bufs=1) as wpool,
            tc.tile_pool(name="stream", bufs=3) as spool,
            tc.tile_pool(name="psum", bufs=1, space="PSUM") as ppool,
        ):
            # ---------------- constants needed early -----------------
            ident = cpool.tile([128, 128], F32, tag="ident")
            nc.sync.dma_start(ident, ident_d[:, :])
            ones1f = cpool.tile([1, 128], F32, tag="ones1f")
            nc.vector.memset(ones1f, 1.0)
            ones1 = cpool.tile([1, 128], WDT, tag="ones1")
            if FP32R:
                nc.vector.tensor_copy(ones1, ones1f)
            else:
                nc.vector.tensor_copy(ones1, ones1f)

            # ---------------- span / validity masks -----------------
            vm_i = wpool.tile([BL, L], I32, tag="vm_i")
            nc.sync.dma_start(vm_i, vmask[:, :])
            vm = wpool.tile([BL, L], F32, tag="vm")
            nc.vector.tensor_copy(vm, vm_i)
            zer = wpool.tile([BL, L], F32, tag="zer")
            nc.vector.memset(zer, 0.0)
            incl = wpool.tile([BL, L], F32, tag="incl")
            nc.vector.tensor_tensor_scan(incl, vm, zer, 0.0, OP.add, OP.add)
            rank = wpool.tile([BL, L], F32, tag="rank")
            nc.vector.tensor_sub(rank, incl, vm)

            ps_i = wpool.tile([BL, 2], I32, tag="ps_i")
            nc.sync.dma_start(ps_i, pspan[:, :])
            psf = wpool.tile([BL, 2], F32, tag="psf")
            nc.vector.tensor_copy(psf, ps_i)

            ge = wpool.tile([BL, L], F32, tag="ge")
            nc.vector.tensor_scalar(ge, rank, psf[:, 0:1], None, OP.is_ge)
            le = wpool.tile([BL, L], F32, tag="le")
            nc.vector.tensor_scalar(le, rank, psf[:, 1:2], None, OP.is_le)
            nc.vector.tensor_mul(ge, ge, le)
            nc.vector.tensor_mul(ge, ge, vm)
            am = wpool.tile([BL, L], F32, tag="am")
            nc.vector.tensor_scalar(am, ge, 1.0, BIG, OP.subtract, OP.mult)

            # zmask: 0 if span reaches past the valid count, else -BIG
            gz = wpool.tile([BL, 1], F32, tag="gz")
            nc.vector.tensor_tensor(gz, psf[:, 1:2], incl[:, L - 1 : L], OP.is_ge)
            zm = wpool.tile([BL, 1], F32, tag="zm")
            nc.vector.tensor_scalar(zm, gz, 1.0, BIG, OP.subtract, OP.mult)

            # PE warm-up: observe the ident DMA lane first.
            ptw = ppool.tile([2, 2], F32, tag="misc", bufs=2)
            nc.tensor.transpose(ptw, ident[:2, :2], ident[:2, :2])

            # transpose additive mask to [token, row] columns
            amT = []
            for c in range(2):
                pt = ppool.tile([128, BL], F32, tag="misc", bufs=2, name=f"ptam{c}")
                nc.tensor.transpose(pt, am[:, c * 128 : (c + 1) * 128], ident[:BL, :BL])
                s = cpool.tile([128, BL], F32, tag=f"amT{c}", name=f"amT{c}")
                nc.vector.tensor_copy(s, pt)
                amT.append(s)

            # ---------------- stream tokens, masked max-pool -----------------
            # Per row: mask+chunk-max in [token, d] layout, then fold the
            # 128 token-partitions into the free dim via a DVE 32x32 stream
            # transpose + 3D-AP reduce, one small PE transpose, a second
            # reduce, and a tiny flatten DMA into the natural feat row.
            fn = wpool.tile([BL, D], F32, tag="fn")
            for b in range(BL):
                e0 = spool.tile([128, D], F32, tag="e0", bufs=4)
                e1 = spool.tile([128, D], F32, tag="e1", bufs=4)
                nc.sync.dma_start(e0, enc[b, 0:128, :])
                nc.sync.dma_start(e1, enc[b, 128:256, :])
                # e1 += mask1 on ACT (engine balance), then fused (e0+mask0) max e1
                nc.scalar.activation(e1, e1, AF.Identity, bias=amT[1][:, b : b + 1])
                nc.vector.scalar_tensor_tensor(
                    e0, e0, amT[0][:, b : b + 1], e1, OP.add, OP.max
                )
                # fused 32x32 block transpose + max over jr: red[32jb+dr, db]
                red = spool.tile([128, 32], F32, tag="red", bufs=4, name="red")
                nc.vector.tensor_reduce(
                    red, e0.rearrange("p (db jr) -> p db jr", jr=32),
                    axis=AX.X, op=OP.max, apply_transpose=True,
                )
                # pred[db, 32jb+dr]
                pred = ppool.tile([32, 128], F32, tag="tr", bufs=4, name="pred")
                nc.tensor.transpose(pred, red, ident)
                # max over jb: mT[db, dr] = feat_b[32db+dr]
                mT = spool.tile([32, 32], F32, tag="mT", bufs=4, name="mT")
                nc.vector.tensor_reduce(
                    mT, pred.rearrange("p (jb dr) -> p dr jb", dr=32),
                    axis=AX.X, op=OP.max,
                )
                # flatten [32db, 32dr] -> natural feat row [1, 1024]
                nc.gpsimd.dma_start(
                    fn[b : b + 1, :].rearrange("a (db dr) -> a db dr", dr=32), mT
                )

            # ---------------- weights / topic constants (after stream) -------
            wt = []
            for i in range(8):
                nk = PCH[i] // 128
                no = PCH[i + 1]
                w = cpool.tile([128, nk * no], WDT, tag=f"w{i}", name=f"wt{i}")
                nc.sync.dma_start(
                    w.rearrange("p (c n) -> p c n", n=no),
                    _w(w_dram[i].rearrange("(c p) n -> p c n", p=128)),
                )
                wt.append(w)
            bt = {}
            for i in (0, 1, 2, 4, 5, 6):
                nm = PCH[i + 1] // 128
                t = cpool.tile([128, nm], F32, tag=f"b{i}", name=f"bt{i}")
                nc.sync.dma_start(t, b_dram[i].rearrange("(c p) -> p c", p=128))
                bt[i] = t
            brow = {}
            for i in (3, 7):
                no = PCH[i + 1]
                t = cpool.tile([1, no], WDT, tag=f"br{i}", name=f"brow{i}")
                nc.sync.dma_start(t, _w(b_dram[i].rearrange("(a n) -> a n", a=1)))
                brow[i] = t
            temb = cpool.tile([KTOP, 100], F32, tag="temb")
            nc.sync.dma_start(temb, temb_d[:, :])

            # ---------------- feat natural: clamp, l2norm ---------
            nc.vector.tensor_scalar(fn, fn, zm[:, 0:1], None, OP.max)
            fsq = wpool.tile([BL, D], F32, tag="fsq")
            n2 = wpool.tile([BL, 1], F32, tag="n2")
            nc.vector.tensor_mul(fsq, fn, fn)
            nc.vector.tensor_reduce(n2, fsq, axis=AX.X, op=OP.add)
            nr = wpool.tile([BL, 1], F32, tag="nr")
            nc.scalar.sqrt(nr, n2)
            nc.vector.tensor_scalar_max(nr, nr, EPS)
            inv = wpool.tile([BL, 1], F32, tag="inv")
            nc.vector.reciprocal(inv, nr)
            xnat = wpool.tile([BL, D], F32, tag="xnat")
            nc.vector.tensor_scalar_mul(xnat, fn, inv[:, 0:1])

            # x^T chunks for the MLP (stationary operand)
            actT = []
            for m in range(8):
                px = ppool.tile([128, BL], F32, tag="misc", bufs=2, name="pxT")
                nc.tensor.transpose(px, xnat[:, m * 128 : (m + 1) * 128], ident[:BL, :BL])
                s = spool.tile([128, BL], WDT, tag="actT_x", bufs=8, name=f"xTn{m}")
                nc.vector.tensor_copy(s, px)
                actT.append(s)

            # ---------------- autoencoder MLP (A-chain) -----------------
            znat = None
            hnat = None
            for i in range(8):
                nk = PCH[i] // 128
                no = PCH[i + 1]
                nat = wpool.tile([BL, no], F32, tag=f"nat{i}", name=f"nat{i}")
                for j in range((no + 511) // 512):
                    w_cols = min(512, no - j * 512)
                    ps = ppool.tile([BL, w_cols], F32, tag="mlp", bufs=2, name=f"psl{i}_{j}")
                    for c in range(nk):
                        o = c * no + j * 512
                        nc.tensor.matmul(
                            ps, actT[c], wt[i][:, o : o + w_cols],
                            start=(c == 0), stop=(c == nk - 1 and i not in (3, 7)),
                        )
                    if i in (3, 7):
                        # bias via K=1 ones-row matmul (natural-exit layers)
                        nc.tensor.matmul(
                            ps, ones1[:, :BL],
                            brow[i][:, j * 512 : j * 512 + w_cols],
                            start=False, stop=True,
                        )
                    nc.vector.tensor_copy(nat[:, j * 512 : j * 512 + w_cols], ps)
                if i == 3:
                    znat = nat
                if i == 7:
                    hnat = nat
                    break  # no transpose needed after the last layer
                nxt = []
                for c in range(no // 128):
                    pT = ppool.tile([128, BL], F32, tag="tr", bufs=4, name=f"pT{i}_{c}")
                    nc.tensor.transpose(
                        pT, nat[:, c * 128 : (c + 1) * 128], ident[:BL, :BL]
                    )
                    oa = spool.tile(
                        [128, BL], WDT, tag=f"actT{i}", bufs=no // 128, name=f"a{i}_{c}"
                    )
                    if i in (3, 7):
                        nc.vector.tensor_copy(oa, pT)
                    else:
                        nc.scalar.activation(oa, pT, AF.Relu, bias=bt[i][:, c : c + 1])
                    nxt.append(oa)
                actT = nxt

            # ---------------- cluster softmax head -----------------
            z2 = wpool.tile([BL, 1], F32, tag="z2")
            zsq = wpool.tile([BL, PCH[4]], F32, tag="zsq")
            nc.vector.tensor_mul(zsq, znat, znat)
            nc.vector.tensor_reduce(z2, zsq, axis=AX.X, op=OP.add)
            znr = wpool.tile([BL, 1], F32, tag="znr")
            nc.scalar.sqrt(znr, z2)
            nc.vector.tensor_scalar_max(znr, znr, EPS)
            zinv = wpool.tile([BL, 1], F32, tag="zinv")
            nc.vector.reciprocal(zinv, znr)
            zn = wpool.tile([BL, PCH[4]], F32, tag="zn")
            nc.vector.tensor_scalar_mul(zn, znat, zinv[:, 0:1])
            pzT = ppool.tile([PCH[4], BL], F32, tag="misc", bufs=2)
            nc.tensor.transpose(pzT, zn, ident[:BL, :BL])
            znT = wpool.tile([PCH[4], BL], F32, tag="znT")
            nc.vector.tensor_copy(znT, pzT)

            tsq = wpool.tile([KTOP, 100], F32, tag="tsq")
            nc.vector.tensor_mul(tsq, temb, temb)
            tn2 = wpool.tile([KTOP, 1], F32, tag="tn2")
            nc.vector.tensor_reduce(tn2, tsq, axis=AX.X, op=OP.add)
            tnr = wpool.tile([KTOP, 1], F32, tag="tnr")
            nc.scalar.sqrt(tnr, tn2)
            nc.vector.tensor_scalar_max(tnr, tnr, EPS)
            tiv = wpool.tile([KTOP, 1], F32, tag="tiv")
            nc.vector.reciprocal(tiv, tnr)
            tn = wpool.tile([KTOP, 100], F32, tag="tn")
            nc.vector.tensor_scalar_mul(tn, temb, tiv)
            ptt = ppool.tile([100, KTOP], F32, tag="misc", bufs=2)
            nc.tensor.transpose(ptt, tn, ident[:KTOP, :KTOP])
            tnT = wpool.tile([100, KTOP], F32, tag="tnT")
            nc.vector.tensor_copy(tnT, ptt)

            psl = ppool.tile([BL, KTOP], F32, tag="misc", bufs=2)
            nc.tensor.matmul(psl, znT[0:100, :], tnT, start=True, stop=True)
            mx = wpool.tile([BL, 1], F32, tag="mx")
            nc.vector.tensor_reduce(mx, psl, axis=AX.X, op=OP.max)
            mxs = wpool.tile([BL, 1], F32, tag="mxs")
            nc.vector.tensor_scalar_mul(mxs, mx, -1.0 / TEMP)
            ex = wpool.tile([BL, KTOP], F32, tag="ex")
            nc.scalar.activation(ex, psl, AF.Exp, bias=mxs, scale=1.0 / TEMP)
            sm = wpool.tile([BL, 1], F32, tag="sm")
            nc.vector.tensor_reduce(sm, ex, axis=AX.X, op=OP.add)
            rs = wpool.tile([BL, 1], F32, tag="rs")
            nc.vector.reciprocal(rs, sm)
            pp = wpool.tile([BL, KTOP], F32, tag="pp")
            nc.vector.tensor_scalar_mul(pp, ex, rs)
            nc.sync.dma_start(p_out[:, :], pp)

            # ---------------- cosine pretrain loss -----------------
            hx = wpool.tile([BL, D], F32, tag="hx")
            s1 = wpool.tile([BL, 1], F32, tag="s1")
            nc.vector.tensor_mul(hx, hnat, xnat)
            nc.vector.tensor_reduce(s1, hx, axis=AX.X, op=OP.add)
            hh = wpool.tile([BL, D], F32, tag="hh")
            n2h = wpool.tile([BL, 1], F32, tag="n2h")
            nc.vector.tensor_mul(hh, hnat, hnat)
            nc.vector.tensor_reduce(n2h, hh, axis=AX.X, op=OP.add)
            hnr = wpool.tile([BL, 1], F32, tag="hnr")
            nc.scalar.sqrt(hnr, n2h)
            nc.vector.tensor_scalar_max(hnr, hnr, EPS)
            hinv = wpool.tile([BL, 1], F32, tag="hinv")
            nc.vector.reciprocal(hinv, hnr)
            ct = wpool.tile([BL, 1], F32, tag="ct")
            nc.vector.tensor_mul(ct, s1, hinv)
            cosd = wpool.tile([BL, 1], F32, tag="cosd")
            nc.vector.tensor_scalar(cosd, ct, -1.0, 1.0, OP.mult, OP.add)
            nc.sync.dma_start(cos_out[:, :], cosd)

    nc.compile()
    return nc


def _pad_weights(enc_W, enc_b, dec_W, dec_b):
    Ws = [np.asarray(w, np.float32) for w in list(enc_W) + list(dec_W)]
    bs = [np.asarray(b, np.float32) for b in list(enc_b) + list(dec_b)]
    pw, pb = [], []
    for i in range(8):
        w = np.zeros((PCH[i], PCH[i + 1]), np.float32)
        w[: Ws[i].shape[0], : Ws[i].shape[1]] = Ws[i]
        b = np.zeros((PCH[i + 1],), np.float32)
        b[: bs[i].shape[0]] = bs[i]
        pw.append(w)
        pb.append(b)
    return pw, pb


def kernel(encoder_layers, valid_mask, pos_span, mask_span,
           enc_W, enc_b, dec_W, dec_b, topic_emb):
    global LAST_EXEC_NS
    enc = np.ascontiguousarray(np.asarray(encoder_layers, np.float32))
    vm = np.ascontiguousarray(np.asarray(valid_mask, np.int32))
    ps = np.ascontiguousarray(np.asarray(pos_span, np.int32))
    te = np.ascontiguousarray(np.asarray(topic_emb, np.float32))
    pw, pb = _pad_weights(enc_W, enc_b, dec_W, dec_b)
    ident = np.eye(128, dtype=np.float32)

    nc = build_module()

    in_maps = []
    for c in range(NCORES):
        sl = slice(c * BL, (c + 1) * BL)
        m = {"enc": enc[sl], "vmask": vm[sl], "pspan": ps[sl],
             "temb": te, "ident": ident}
        for i in range(8):
            m[f"w{i}"] = pw[i]
            m[f"b{i}"] = pb[i]
        in_maps.append(m)

    res = run_bass_kernel_spmd(
        nc, in_maps, core_ids=list(range(NCORES)), trace=TRACE,
    )
    LAST_EXEC_NS = res.exec_time_ns

    p = np.concatenate([r["p_out"] for r in res.results], axis=0)
    cos = np.concatenate([r["cos_out"] for r in res.results], axis=0).reshape(-1)
    loss = np.float32(cos.mean())
    return loss, p.astype(np.float32)


# revision 35
# speedup vs baseline: 1.2593x; 1.0194x over previous
"""Trainium2 Bass kernel for nn_ETypeClusModel (ragged span max-pool + AE + cluster softmax).

Self-contained: hardcodes shapes. Shards the batch dim over 8 NeuronCores
(data parallel), replicates the AE weights + topic embedding, and gathers
p / per-row cosine distances back to the host (loss = mean of 128 values).

Layout strategy (per core, 16 batch rows):
 - span max-pool: stream token rows [128, 1024] (two 512KB DMAs/row),
   additive -1e30 span masks via per-partition scalars (one on ACT, one
   fused into the DVE chunk-pair max), then fold the 128 token-partitions
   into the free dim with a single DVE reduce_max using apply_transpose
   (fused 32x32 stream transpose), one small PE transpose, a second
   reduce, and a tiny flatten DMA into the natural [row, d] feat tile.
 - zero-clamp, l2-norm, zn/topic norms, softmax, and the cosine head are
   all free-dim / per-partition ops in natural layout.
 - MLP runs with activations as the stationary operand (lhsT [128, 16])
   and weights as the moving operand (N up to 512) in float32r (full-rate
   PE, ~19-bit mantissa; end-to-end rel err ~4e-4). Bias+ReLU ride the
   inter-layer transposes on the scalar engine (per-partition bias);
   natural-exit layers (z, x_bar) get their bias via a K=1 ones-row
   matmul accumulated into PSUM.
 - AE weights are zero-padded to multiples of 128 on the host and
   DMA'd after the token stream (enc-row cadence paces the DVE pipeline;
   measured faster than weights-first or interleaved orders).
"""

import os
import numpy as np

import concourse.bacc as bacc
import concourse.bass as bass
import concourse.mybir as mybir
import concourse.tile as tile
from concourse.bass_utils import run_bass_kernel_spmd

F32 = mybir.dt.float32
I32 = mybir.dt.int32
AF = mybir.ActivationFunctionType
OP = mybir.AluOpType
AX = mybir.AxisListType

B, L, D = 128, 256, 1024
NCORES = 8
BL = B // NCORES  # 16 rows per core
KTOP = 50
TEMP = 0.1
EPS = 1e-12
BIG = 1.0e30

# layer chain 1024->500->500->1000->100->1000->500->500->1024, padded to /128
CHAIN = [1024, 500, 500, 1000, 100, 1000, 500, 500, 1024]
PCH = [1024, 512, 512, 1024, 128, 1024, 512, 512, 1024]
RELU = [True, True, True, False, True, True, True, False]

LAST_EXEC_NS = None  # set after each kernel() call when tracing enabled
TRACE = bool(int(os.environ.get("BASS_KERNEL_TRACE", "0")))
FP32R = bool(int(os.environ.get("BASS_FP32R", "1")))


WDT = mybir.dt.float32r if FP32R else mybir.dt.float32


def _w(ap):
    """Bitcast a DRAM fp32 AP to the MLP operand dtype (float32r mode)."""
    return ap.bitcast(WDT) if FP32R else ap


def build_module():
    nc = bacc.Bacc("TRN2", target_bir_lowering=False)

    enc = nc.dram_tensor("enc", (BL, L, D), F32, kind="ExternalInput")
    vmask = nc.dram_tensor("vmask", (BL, L), I32, kind="ExternalInput")
    pspan = nc.dram_tensor("pspan", (BL, 2), I32, kind="ExternalInput")
    w_dram = [
        nc.dram_tensor(f"w{i}", (PCH[i], PCH[i + 1]), F32, kind="ExternalInput")
        for i in range(8)
    ]
    b_dram = [
        nc.dram_tensor(f"b{i}", (PCH[i + 1],), F32, kind="ExternalInput")
        for i in range(8)
    ]
    temb_d = nc.dram_tensor("temb", (KTOP, 100), F32, kind="ExternalInput")
    ident_d = nc.dram_tensor("ident", (128, 128), F32, kind="ExternalInput")

    p_out = nc.dram_tensor("p_out", (BL, KTOP), F32, kind="ExternalOutput")
    cos_out = nc.dram_tensor("cos_out", (BL, 1), F32, kind="ExternalOutput")

    with tile.TileContext(nc) as tc:
        with (
            tc.tile_pool(name="const", bufs=1) as cpool,
            tc.tile_pool(name="work", bufs=1) as wpool,
            tc.tile_pool(name="stream", bufs=3) as spool,
            tc.tile_pool(name="psum", bufs=1, space="PSUM") as ppool,
        ):
            # ---------------- constants needed early -----------------
            ident = cpool.tile([128, 128], F32, tag="ident")
            nc.sync.dma_start(ident, ident_d[:, :])
            identR = cpool.tile([128, 128], WDT, tag="identR")
            nc.vector.tensor_copy(identR, ident)
            ones1f = cpool.tile([1, 128], F32, tag="ones1f")
            nc.vector.memset(ones1f, 1.0)
            ones1 = cpool.tile([1, 128], WDT, tag="ones1")
            if FP32R:
                nc.vector.tensor_copy(ones1, ones1f)
            else:
                nc.vector.tensor_copy(ones1, ones1f)

            # ---------------- span / validity masks -----------------
            vm_i = wpool.tile([BL, L], I32, tag="vm_i")
            nc.sync.dma_start(vm_i, vmask[:, :])
            vm = wpool.tile([BL, L], F32, tag="vm")
            nc.vector.tensor_copy(vm, vm_i)
            zer = wpool.tile([BL, L], F32, tag="zer")
            nc.vector.memset(zer, 0.0)
            incl = wpool.tile([BL, L], F32, tag="incl")
            nc.vector.tensor_tensor_scan(incl, vm, zer, 0.0, OP.add, OP.add)
            rank = wpool.tile([BL, L], F32, tag="rank")
            nc.vector.tensor_sub(rank, incl, vm)

            ps_i = wpool.tile([BL, 2], I32, tag="ps_i")
            nc.sync.dma_start(ps_i, pspan[:, :])
            psf = wpool.tile([BL, 2], F32, tag="psf")
            nc.vector.tensor_copy(psf, ps_i)

            ge = wpool.tile([BL, L], F32, tag="ge")
            nc.vector.tensor_scalar(ge, rank, psf[:, 0:1], None, OP.is_ge)
            le = wpool.tile([BL, L], F32, tag="le")
            nc.vector.tensor_scalar(le, rank, psf[:, 1:2], None, OP.is_le)
            nc.vector.tensor_mul(ge, ge, le)
            nc.vector.tensor_mul(ge, ge, vm)
            am = wpool.tile([BL, L], F32, tag="am")
            nc.vector.tensor_scalar(am, ge, 1.0, BIG, OP.subtract, OP.mult)

            # zmask: 0 if span reaches past the valid count, else -BIG
            gz = wpool.tile([BL, 1], F32, tag="gz")
            nc.vector.tensor_tensor(gz, psf[:, 1:2], incl[:, L - 1 : L], OP.is_ge)
            zm = wpool.tile([BL, 1], F32, tag="zm")
            nc.vector.tensor_scalar(zm, gz, 1.0, BIG, OP.subtract, OP.mult)

            # PE warm-up: observe the ident DMA lane first.
            ptw = ppool.tile([2, 2], F32, tag="misc", bufs=2)
            nc.tensor.transpose(ptw, ident[:2, :2], ident[:2, :2])

            # transpose additive mask to [token, row] columns
            amT = []
            for c in range(2):
                pt = ppool.tile([128, BL], F32, tag="misc", bufs=2, name=f"ptam{c}")
                nc.tensor.transpose(pt, am[:, c * 128 : (c + 1) * 128], ident[:BL, :BL])
                s = cpool.tile([128, BL], F32, tag=f"amT{c}", name=f"amT{c}")
                nc.vector.tensor_copy(s, pt)
                amT.append(s)

            # ---------------- stream tokens, masked max-pool -----------------
            # Per row: mask+chunk-max in [token, d] layout, then fold the
            # 128 token-partitions into the free dim via a DVE 32x32 stream
            # transpose + 3D-AP reduce, one small PE transpose, a second
            # reduce, and a tiny flatten DMA into the natural feat row.
            fn = wpool.tile([BL, D], F32, tag="fn")
            for b in range(BL):
                e0 = spool.tile([128, D], F32, tag="e0", bufs=4)
                e1 = spool.tile([128, D], F32, tag="e1", bufs=4)
                nc.sync.dma_start(e0, enc[b, 0:128, :])
                nc.sync.dma_start(e1, enc[b, 128:256, :])
                # e1 += mask1 on ACT (engine balance), then fused (e0+mask0) max e1
                nc.scalar.activation(e1, e1, AF.Identity, bias=amT[1][:, b : b + 1])
                nc.vector.scalar_tensor_tensor(
                    e0, e0, amT[0][:, b : b + 1], e1, OP.add, OP.max
                )
                # fused 32x32 block transpose + max over jr: red[32jb+dr, db]
                red = spool.tile([128, 32], F32, tag="red", bufs=4, name="red")
                nc.vector.tensor_reduce(
                    red, e0.rearrange("p (db jr) -> p db jr", jr=32),
                    axis=AX.X, op=OP.max, apply_transpose=True,
                )
                # pred[db, 32jb+dr]
                pred = ppool.tile([32, 128], F32, tag="tr", bufs=4, name="pred")
                nc.tensor.transpose(pred, red, ident)
                # max over jb: mT[db, dr] = feat_b[32db+dr]
                mT = spool.tile([32, 32], F32, tag="mT", bufs=4, name="mT")
                nc.vector.tensor_reduce(
                    mT, pred.rearrange("p (jb dr) -> p dr jb", dr=32),
                    axis=AX.X, op=OP.max,
                )
                # flatten [32db, 32dr] -> natural feat row [1, 1024]
                nc.gpsimd.dma_start(
                    fn[b : b + 1, :].rearrange("a (db dr) -> a db dr", dr=32), mT
                )

            # ---------------- weights / topic constants (after stream) -------
            wt = []
            for i in range(8):
                nk = PCH[i] // 128
                no = PCH[i + 1]
                w = cpool.tile([128, nk * no], WDT, tag=f"w{i}", name=f"wt{i}")
                nc.sync.dma_start(
                    w.rearrange("p (c n) -> p c n", n=no),
                    _w(w_dram[i].rearrange("(c p) n -> p c n", p=128)),
                )
                wt.append(w)
            bt = {}
            for i in (0, 1, 2, 4, 5, 6):
                nm = PCH[i + 1] // 128
                t = cpool.tile([128, nm], F32, tag=f"b{i}", name=f"bt{i}")
                nc.sync.dma_start(t, b_dram[i].rearrange("(c p) -> p c", p=128))
                bt[i] = t
            brow = {}
            for i in (3, 7):
                no = PCH[i + 1]
                t = cpool.tile([1, no], WDT, tag=f"br{i}", name=f"brow{i}")
                nc.sync.dma_start(t, _w(b_dram[i].rearrange("(a n) -> a n", a=1)))
                brow[i] = t
            temb = cpool.tile([KTOP, 100], F32, tag="temb")
            nc.sync.dma_start(temb, temb_d[:, :])

            # ---------------- feat natural: clamp, l2norm ---------
            nc.vector.tensor_scalar(fn, fn, zm[:, 0:1], None, OP.max)
            fsq = wpool.tile([BL, D], F32, tag="fsq")
            n2 = wpool.tile([BL, 1], F32, tag="n2")
            nc.vector.tensor_mul(fsq, fn, fn)
            nc.vector.tensor_reduce(n2, fsq, axis=AX.X, op=OP.add)
            nr = wpool.tile([BL, 1], F32, tag="nr")
            nc.scalar.sqrt(nr, n2)
            nc.vector.tensor_scalar_max(nr, nr, EPS)
            inv = wpool.tile([BL, 1], F32, tag="inv")
            nc.vector.reciprocal(inv, nr)
            xnat = wpool.tile([BL, D], F32, tag="xnat")
            nc.vector.tensor_scalar_mul(xnat, fn, inv[:, 0:1])

            # x^T chunks for the MLP (stationary operand)
            actT = []
            for m in range(8):
                px = ppool.tile([128, BL], F32, tag="misc", bufs=2, name="pxT")
                nc.tensor.transpose(px, xnat[:, m * 128 : (m + 1) * 128], ident[:BL, :BL])
                s = spool.tile([128, BL], WDT, tag="actT_x", bufs=8, name=f"xTn{m}")
                nc.vector.tensor_copy(s, px)
                actT.append(s)

            # ---------------- autoencoder MLP (A-chain) -----------------
            znat = None
            hnat = None
            for i in range(8):
                nk = PCH[i] // 128
                no = PCH[i + 1]
                ndt = F32 if i in (3, 7) else WDT
                nat = wpool.tile([BL, no], ndt, tag=f"nat{i}", name=f"nat{i}")
                for j in range((no + 511) // 512):
                    w_cols = min(512, no - j * 512)
                    ps = ppool.tile([BL, w_cols], F32, tag="mlp", bufs=2, name=f"psl{i}_{j}")
                    for c in range(nk):
                        o = c * no + j * 512
                        nc.tensor.matmul(
                            ps, actT[c], wt[i][:, o : o + w_cols],
                            start=(c == 0), stop=(c == nk - 1 and i not in (3, 7)),
                        )
                    if i in (3, 7):
                        # bias via K=1 ones-row matmul (natural-exit layers)
                        nc.tensor.matmul(
                            ps, ones1[:, :BL],
                            brow[i][:, j * 512 : j * 512 + w_cols],
                            start=False, stop=True,
                        )
                    nc.vector.tensor_copy(nat[:, j * 512 : j * 512 + w_cols], ps)
                if i == 3:
                    znat = nat
                if i == 7:
                    hnat = nat
                    break  # no transpose needed after the last layer
                nxt = []
                for c in range(no // 128):
                    pT = ppool.tile([128, BL], ndt, tag="tr", bufs=4, name=f"pT{i}_{c}")
                    nc.tensor.transpose(
                        pT, nat[:, c * 128 : (c + 1) * 128],
                        (ident if i in (3, 7) else identR)[:BL, :BL],
                    )
                    oa = spool.tile(
                        [128, BL], WDT, tag=f"actT{i}", bufs=no // 128, name=f"a{i}_{c}"
                    )
                    if i in (3, 7):
                        nc.vector.tensor_copy(oa, pT)
                    else:
                        nc.scalar.activation(oa, pT, AF.Relu, bias=bt[i][:, c : c + 1])
                    nxt.append(oa)
                actT = nxt

            # ---------------- cluster softmax head -----------------
            z2 = wpool.tile([BL, 1], F32, tag="z2")
            zsq = wpool.tile([BL, PCH[4]], F32, tag="zsq")
            nc.vector.tensor_mul(zsq, znat, znat)
            nc.vector.tensor_reduce(z2, zsq, axis=AX.X, op=OP.add)
            znr = wpool.tile([BL, 1], F32, tag="znr")
            nc.scalar.sqrt(znr, z2)
            nc.vector.tensor_scalar_max(znr, znr, EPS)
            zinv = wpool.tile([BL, 1], F32, tag="zinv")
            nc.vector.reciprocal(zinv, znr)
            zn = wpool.tile([BL, PCH[4]], F32, tag="zn")
            nc.vector.tensor_scalar_mul(zn, znat, zinv[:, 0:1])
            pzT = ppool.tile([PCH[4], BL], F32, tag="misc", bufs=2)
            nc.tensor.transpose(pzT, zn, ident[:BL, :BL])
            znT = wpool.tile([PCH[4], BL], F32, tag="znT")
            nc.vector.tensor_copy(znT, pzT)

            tsq = wpool.tile([KTOP, 100], F32, tag="tsq")
            nc.vector.tensor_mul(tsq, temb, temb)
            tn2 = wpool.tile([KTOP, 1], F32, tag="tn2")
            nc.vector.tensor_reduce(tn2, tsq, axis=AX.X, op=OP.add)
            tnr = wpool.tile([KTOP, 1], F32, tag="tnr")
            nc.scalar.sqrt(tnr, tn2)
            nc.vector.tensor_scalar_max(tnr, tnr, EPS)
            tiv = wpool.tile([KTOP, 1], F32, tag="tiv")
            nc.vector.reciprocal(tiv, tnr)
            tn = wpool.tile([KTOP, 100], F32, tag="tn")
            nc.vector.tensor_scalar_mul(tn, temb, tiv)
            ptt = ppool.tile([100, KTOP], F32, tag="misc", bufs=2)
            nc.tensor.transpose(ptt, tn, ident[:KTOP, :KTOP])
            tnT = wpool.tile([100, KTOP], F32, tag="tnT")
            nc.vector.tensor_copy(tnT, ptt)

            psl = ppool.tile([BL, KTOP], F32, tag="misc", bufs=2)
            nc.tensor.matmul(psl, znT[0:100, :], tnT, start=True, stop=True)
            mx = wpool.tile([BL, 1], F32, tag="mx")
            nc.vector.tensor_reduce(mx, psl, axis=AX.X, op=OP.max)
            mxs = wpool.tile([BL, 1], F32, tag="mxs")
            nc.vector.tensor_scalar_mul(mxs, mx, -1.0 / TEMP)
            ex = wpool.tile([BL, KTOP], F32, tag="ex")
            nc.scalar.activation(ex, psl, AF.Exp, bias=mxs, scale=1.0 / TEMP)
            sm = wpool.tile([BL, 1], F32, tag="sm")
            nc.vector.tensor_reduce(sm, ex, axis=AX.X, op=OP.add)
            rs = wpool.tile([BL, 1], F32, tag="rs")
            nc.vector.reciprocal(rs, sm)
            pp = wpool.tile([BL, KTOP], F32, tag="pp")
            nc.vector.tensor_scalar_mul(pp, ex, rs)
            nc.sync.dma_start(p_out[:, :], pp)

            # ---------------- cosine pretrain loss -----------------
            hx = wpool.tile([BL, D], F32, tag="hx")
            s1 = wpool.tile([BL, 1], F32, tag="s1")
            nc.vector.tensor_mul(hx, hnat, xnat)
            nc.vector.tensor_reduce(s1, hx, axis=AX.X, op=OP.add)
            hh = wpool.tile([BL, D], F32, tag="hh")
            n2h = wpool.tile([BL, 1], F32, tag="n2h")
            nc.vector.tensor_mul(hh, hnat, hnat)
            nc.vector.tensor_reduce(n2h, hh, axis=AX.X, op=OP.add)
            hnr = wpool.tile([BL, 1], F32, tag="hnr")
            nc.scalar.sqrt(hnr, n2h)
            nc.vector.tensor_scalar_max(hnr, hnr, EPS)
            hinv = wpool.tile([BL, 1], F32, tag="hinv")
            nc.vector.reciprocal(hinv, hnr)
            ct = wpool.tile([BL, 1], F32, tag="ct")
            nc.vector.tensor_mul(ct, s1, hinv)
            cosd = wpool.tile([BL, 1], F32, tag="cosd")
            nc.vector.tensor_scalar(cosd, ct, -1.0, 1.0, OP.mult, OP.add)
            nc.sync.dma_start(cos_out[:, :], cosd)

    nc.compile()
    return nc


def _pad_weights(enc_W, enc_b, dec_W, dec_b):
    Ws = [np.asarray(w, np.float32) for w in list(enc_W) + list(dec_W)]
    bs = [np.asarray(b, np.float32) for b in list(enc_b) + list(dec_b)]
    pw, pb = [], []
    for i in range(8):
        w = np.zeros((PCH[i], PCH[i + 1]), np.float32)
        w[: Ws[i].shape[0], : Ws[i].shape[1]] = Ws[i]
        b = np.zeros((PCH[i + 1],), np.float32)
        b[: bs[i].shape[0]] = bs[i]
        pw.append(w)
        pb.append(b)
    return pw, pb


def kernel(encoder_layers, valid_mask, pos_span, mask_span,
           enc_W, enc_b, dec_W, dec_b, topic_emb):
    global LAST_EXEC_NS
    enc = np.ascontiguousarray(np.asarray(encoder_layers, np.float32))
    vm = np.ascontiguousarray(np.asarray(valid_mask, np.int32))
    ps = np.ascontiguousarray(np.asarray(pos_span, np.int32))
    te = np.ascontiguousarray(np.asarray(topic_emb, np.float32))
    pw, pb = _pad_weights(enc_W, enc_b, dec_W, dec_b)
    ident = np.eye(128, dtype=np.float32)

    nc = build_module()

    in_maps = []
    for c in range(NCORES):
        sl = slice(c * BL, (c + 1) * BL)
        m = {"enc": enc[sl], "vmask": vm[sl], "pspan": ps[sl],
             "temb": te, "ident": ident}
        for i in range(8):
            m[f"w{i}"] = pw[i]
            m[f"b{i}"] = pb[i]
        in_maps.append(m)

    res = run_bass_kernel_spmd(
        nc, in_maps, core_ids=list(range(NCORES)), trace=TRACE,
    )
    LAST_EXEC_NS = res.exec_time_ns

    p = np.concatenate([r["p_out"] for r in res.results], axis=0)
    cos = np.concatenate([r["cos_out"] for r in res.results], axis=0).reshape(-1)
    loss = np.float32(cos.mean())
    return loss, p.astype(np.float32)
